# revision 1
# baseline (speedup 1.0000x reference)
"""Trainium2 Bass kernel for nn_Decoder (Bahdanau-attention decoder scan).

Contract: kernel(**inputs) takes FULL unsharded numpy inputs (keys as in
setup_inputs()) and returns the FULL [B, S, O] float32 output.

Sharding: pure data-parallel over batch B across 8 NeuronCores; weights
replicated; scan carry local per core.

Per-core algorithm (B_c = B/8 = 256, T = 2048, U = 16):
  pre-pass (TileContext #1):
    enc fp32 -> fp16; encw1 = enc @ w1 on PE via a block-diag kron(I8, w1)
    stationary against DMA-xbar-transposed tiles; encw1 lands as two
    resident fp16 SBUF tensors [128b, T, U]; enc fp16 echo goes to DRAM
    scratch [b][t][u] for per-step streaming.
  scan (TileContext #2), per step s:
    q       = h_aug.T @ [w2_k; w2_b]                    (PE, tiny)
    tanh_u  = tanh(W[:, :, u] + q[:, u])                (ScalarE, fused bias)
    score   = sum_u w3[u] * tanh_u                      (PE identity-matmuls
                                                         accumulated in PSUM)
    e       = exp(score), sum_e via fused accum_out     (ScalarE; no max-sub
                                                         needed: |score|<~1.3)
    ctx_u   = sum_t e * enc_u / sum_e                   (VectorE
                                                         tensor_tensor_reduce)
    GRU with h0=0 degenerates:  new_h = 0.5*(1-tanh(xz/2))*tanh(xh)
    (sigmoid avoided => single ACT table set for the whole kernel)
    out_s   = new_h_aug.T @ [dense_k; dense_b]          (PE, tiny)

Two TileContexts: the pre-pass exit barrier resets Tile's vector clocks, so
scan-loop matmuls don't inherit one sync-wait per pre-pass DMA HW queue
(the matmul LW instruction has a tiny sync-wait budget in walrus).
Resident/state data lives in raw alloc_sbuf_tensor allocations that
survive across the contexts.
"""

import sys

import numpy as np

sys.path.insert(0, "/opt/trn_rl_repo")

import concourse.bass as bass  # noqa: E402
import concourse.tile as tile  # noqa: E402
from concourse import mybir  # noqa: E402

F32 = mybir.dt.float32
F16 = mybir.dt.float16

# Instructions that never carry lowered sync waits / have no events field.
_MULTIWAIT_OK = {"InstUnconditionalBranch", "InstCall",
                 "InstRegisterMove", "InstRegisterAlu"}


def _legalize_sync_waits(nc, max_waits=1):
    """Walrus' codegen allows very few sync-wait commands per datapath
    instruction (matmul LW and TensorScalarPtr fail at 2). Engine queues
    are strict FIFO, so we can splice single-wait NOPs in front of any
    instruction that carries more than `max_waits` waits and leave only
    the last wait on the instruction itself."""
    k = 0
    for func in nc.m.functions:
        for bb in func.blocks:
            il = bb.instructions
            i = 0
            while i < len(il):
                ins = il[i]
                si = ins.sync_info
                if (type(ins).__name__ not in _MULTIWAIT_OK
                        and si is not None and si.on_wait
                        and len(si.on_wait) > max_waits):
                    waits = list(si.on_wait)
                    for w in waits[:-max_waits]:
                        nop = mybir.InstNoOp(name=f"syncsplit-{k}",
                                             ins=[], outs=[])
                        k += 1
                        nop.engine = ins.engine
                        nop.sync_info = mybir.SyncInfo(on_wait=[w],
                                                       on_update=[])
                        il.insert(i, nop)
                        i += 1
                    ins.sync_info = mybir.SyncInfo(
                        on_wait=waits[-max_waits:],
                        on_update=list(si.on_update or []))
                i += 1
    return k

N_CORES = 8
U = 16
O = 8
P = 128  # partitions
KA = 48  # augmented contraction: row 0 = bias, rows 32:48 = data


def build_program(B_c, T, S, legalize=True):
    """Build the single-core bass program (same program runs SPMD per core)."""
    assert B_c % P == 0 and T % P == 0
    NB = B_c // P
    TLO = 8
    NTHI = T // TLO
    CT = 128  # t-chunk for the fp32 load/convert stage
    CTS = 512 if T >= 512 else T  # t-chunk for per-step enc streaming
    NCH = T // CTS
    NSC = max(1, T // 512)  # score psum 512-column slices
    SCW = min(T, 512)
    AXROWS = 2048
    assert (NTHI * B_c) % AXROWS == 0
    assert (T * U) % AXROWS == 0

    nc = bass.Bass("TRN2", target_bir_lowering=False)

    enc = nc.dram_tensor("enc", [B_c, T, U], F32, kind="ExternalInput").ap()
    hidden = nc.dram_tensor("hidden", [B_c, U], F32, kind="ExternalInput").ap()
    ident16 = nc.dram_tensor("ident16", [P, P], F16, kind="ExternalInput").ap()
    w3ck = nc.dram_tensor("w3ck", [P, U], F32, kind="ExternalInput").ap()
    kronw1 = nc.dram_tensor("kronw1", [P, P], F16, kind="ExternalInput").ap()
    wq = nc.dram_tensor("wq", [KA, U], F32, kind="ExternalInput").ap()
    wg = nc.dram_tensor("wg", [KA, 2 * U], F32, kind="ExternalInput").ap()
    wd = nc.dram_tensor("wd", [KA, O], F32, kind="ExternalInput").ap()
    ident = nc.dram_tensor("ident", [P, P], F32, kind="ExternalInput").ap()
    out = nc.dram_tensor("out", [B_c, S, O], F32, kind="ExternalOutput").ap()

    # DRAM scratch
    tmp = nc.dram_tensor("tmp_bm", [NTHI, B_c, TLO * U], F16).ap()
    w1x = nc.dram_tensor("w1x", [T * U, B_c], F16).ap()
    encsc = nc.dram_tensor("encsc", [B_c, T, U], F16).ap()

    # raw SBUF residents (survive across both TileContexts)
    w1res = [nc.alloc_sbuf_tensor(f"w1res{bt}", [P, T, U], F16).ap()
             for bt in range(NB)]
    w3id_sb = nc.alloc_sbuf_tensor("w3id_r", [P, U, P], F16).ap()
    ident_sb = nc.alloc_sbuf_tensor("ident_r", [P, P], F32).ap()
    wq_sb = nc.alloc_sbuf_tensor("wq_r", [KA, U], F32).ap()
    wg_sb = nc.alloc_sbuf_tensor("wg_r", [KA, 2 * U], F32).ap()
    wd_sb = nc.alloc_sbuf_tensor("wd_r", [KA, O], F32).ap()
    haug = [nc.alloc_sbuf_tensor(f"haug{bt}", [KA, P], F32).ap()
            for bt in range(NB)]

    # ================= TileContext 1: pre-pass =================
    with tile.TileContext(nc) as tc:
        with tc.tile_pool(name="pp_psum", bufs=2, space="PSUM") as pp_psum, \
             tc.tile_pool(name="pp_sbuf", bufs=2) as pp:

            # small weights into residents
            nc.sync.dma_start(out=ident_sb, in_=ident)
            nc.sync.dma_start(out=wq_sb, in_=wq)
            nc.sync.dma_start(out=wg_sb, in_=wg)
            nc.sync.dma_start(out=wd_sb, in_=wd)

            # w3[u]*I stationaries built on-chip (host pre-broadcasts w3
            # down the partitions; a big [U,P,P] DMA would fan out over
            # many HW queues and exceed walrus' per-instruction sync-wait
            # budget on its consumers)
            id16d = pp.tile([P, P], F16, tag="id16d")
            nc.sync.dma_start(out=id16d, in_=ident16)
            w3bd = pp.tile([P, U], F32, tag="w3bd")
            nc.sync.dma_start(out=w3bd, in_=w3ck)
            # route both operands through DVE so the tensor_scalar below has
            # only same-engine deps (its walrus lowering allows 1 sync wait)
            id16 = pp.tile([P, P], F16, tag="id16")
            nc.vector.tensor_copy(id16, id16d)
            w3b = pp.tile([P, U], F32, tag="w3b")
            nc.vector.tensor_copy(w3b, w3bd)
            for u in range(U):
                nc.vector.tensor_scalar(
                    out=w3id_sb[:, u, :], in0=id16,
                    scalar1=w3b[:, u:u + 1], scalar2=None,
                    op0=mybir.AluOpType.mult,
                )

            kron_sb = pp.tile([P, P], F16, tag="kron")
            nc.sync.dma_start(out=kron_sb, in_=kronw1)

            # h_aug init from `hidden` (host pre-scales hidden by -2)
            for bt in range(NB):
                h0 = pp.tile([P, U], F32, tag="h0")
                nc.sync.dma_start(out=h0, in_=hidden[bt * P:(bt + 1) * P, :])
                hT = pp_psum.tile([U, P], F32, tag="hT")
                nc.tensor.transpose(hT, h0, ident_sb)
                nc.vector.memset(haug[bt], 0.0)
                nc.vector.memset(haug[bt][0:1, :], 1.0)
                nc.vector.tensor_copy(haug[bt][32:48, :], hT)

            # phase 1: fp32 load -> fp16 -> tmp [thi][b][tlo*u] (+ encsc echo)
            for bt in range(NB):
                for c in range(T // CT):
                    ld = pp.tile([P, CT, U], F32, tag="pp_ld")
                    nc.sync.dma_start(
                        out=ld,
                        in_=enc[bt * P:(bt + 1) * P, c * CT:(c + 1) * CT, :],
                    )
                    cv = pp.tile([P, CT, U], F16, tag="pp_cv")
                    nc.vector.tensor_copy(cv, ld)
                    nc.scalar.dma_start(
                        out=encsc[bt * P:(bt + 1) * P, c * CT:(c + 1) * CT, :],
                        in_=cv,
                    )
                    tv = tmp.rearrange("thi b i -> b thi i")
                    nthi = CT // TLO
                    nc.sync.dma_start(
                        out=tv[bt * P:(bt + 1) * P,
                               c * nthi:(c + 1) * nthi, :],
                        in_=cv.rearrange("b (thi tlo) u -> b thi (tlo u)",
                                         tlo=TLO),
                    )

            # phase 2: A-xbar transpose + blockdiag matmul -> w1x [(t u), b]
            tmp_rows = tmp.rearrange("thi b i -> (thi b) i")
            w1_v = w1x.rearrange("(thi p) b -> p thi b", p=P)
            nax = (NTHI * B_c) // AXROWS
            thi_per_ax = AXROWS // B_c
            for r in range(nax):
                ax = pp.tile([P, AXROWS], F16, tag="pp_ax")
                nc.sync.dma_start(
                    out=ax,
                    in_=tmp_rows[r * AXROWS:(r + 1) * AXROWS, :],
                    transpose=True,
                )
                evbig = pp.tile([P, AXROWS], F16, tag="pp_ev")
                nmm = AXROWS // 512
                for n in range(nmm):
                    ps = pp_psum.tile([P, 512], F32, tag="pp_bd")
                    nc.tensor.matmul(ps, lhsT=kron_sb,
                                     rhs=ax[:, n * 512:(n + 1) * 512],
                                     start=True, stop=True)
                    if n % 2 == 0:
                        nc.vector.tensor_copy(
                            evbig[:, n * 512:(n + 1) * 512], ps)
                    else:
                        nc.scalar.copy(evbig[:, n * 512:(n + 1) * 512], ps)
                ts = slice(r * thi_per_ax, (r + 1) * thi_per_ax)
                nc.sync.dma_start(
                    out=w1_v[:, ts, :],
                    in_=evbig.rearrange("p (thi b) -> p thi b",
                                        thi=thi_per_ax),
                )

            # phase 3: B-xbar -> resident interleaved encw1 tensors
            for bt in range(NB):
                plv = w1res[bt].rearrange("b t u -> b (t u)")
                for r in range((T * U) // AXROWS):
                    nc.sync.dma_start(
                        out=plv[:, r * AXROWS:(r + 1) * AXROWS],
                        in_=w1x[r * AXROWS:(r + 1) * AXROWS,
                                bt * P:(bt + 1) * P],
                        transpose=True,
                    )

    # ================= TileContext 2: the decoder scan =================
    with tile.TileContext(nc) as tc2:
        with tc2.tile_pool(name="score_psum", bufs=1, space="PSUM") as sps, \
             tc2.tile_pool(name="tiny_psum", bufs=3, space="PSUM") as tps, \
             tc2.tile_pool(name="planes", bufs=2) as planes, \
             tc2.tile_pool(name="stream", bufs=2) as stream, \
             tc2.tile_pool(name="sm", bufs=2) as sm, \
             tc2.tile_pool(name="outp", bufs=1) as outp:

            outacc = []
            for bt in range(NB):
                oa = outp.tile([P, S * O], F32, tag=f"outacc{bt}",
                               name=f"outacc{bt}")
                outacc.append(oa)

            for s in range(S):
                for bt in range(NB):
                    # q = h_prev_aug.T @ [-0.5*w2_k; w2_b]
                    q_ps = tps.tile([P, U], F32, tag="tiny_ps")
                    nc.tensor.matmul(q_ps, lhsT=haug[bt], rhs=wq_sb,
                                     start=True, stop=True)
                    q_sb = sm.tile([P, U], F32, tag="q_sb")
                    nc.vector.tensor_copy(q_sb, q_ps)

                    score = sps.tile([P, T], F32, tag="score")
                    for u in range(U):
                        th = planes.tile([P, T], F16, tag="tanh_plane")
                        nc.scalar.activation(
                            th, w1res[bt][:, :, u],
                            mybir.ActivationFunctionType.Tanh,
                            bias=q_sb[:, u:u + 1], scale=1.0,
                        )
                        for n in range(NSC):
                            nc.tensor.matmul(
                                score[:, n * SCW:(n + 1) * SCW],
                                lhsT=w3id_sb[:, u, :],
                                rhs=th[:, n * SCW:(n + 1) * SCW],
                                start=(u == 0), stop=(u == U - 1),
                            )

                    e_sb = sm.tile([P, T], F16, tag="e_sb")
                    sum_e = sm.tile([P, 1], F32, tag="sum_e")
                    nc.scalar.activation(
                        e_sb, score, mybir.ActivationFunctionType.Exp,
                        accum_out=sum_e,
                    )
                    rs = sm.tile([P, 1], F32, tag="rs")
                    nc.vector.reciprocal(rs, sum_e)

                    # ctx partials via scalar_tensor_tensor's fused
                    # multiply + free-axis sum (tensor_tensor_reduce is an
                    # InstISA whose encoding this walrus rejects)
                    parts = sm.tile([P, NCH, U], F32, tag="parts")
                    for c in range(NCH):
                        ec = stream.tile([P, CTS, U], F16, tag="ec")
                        nc.sync.dma_start(
                            out=ec,
                            in_=encsc[bt * P:(bt + 1) * P,
                                      c * CTS:(c + 1) * CTS, :],
                        )
                        for u in range(U):
                            prod = stream.tile([P, CTS], F16, tag="prod")
                            nc.vector.scalar_tensor_tensor(
                                out=prod,
                                in0=e_sb[:, c * CTS:(c + 1) * CTS],
                                scalar=1.0,
                                in1=ec[:, :, u],
                                op0=mybir.AluOpType.mult,
                                op1=mybir.AluOpType.mult,
                                accum_out=parts[:, c, u:u + 1],
                            )
                    ctxp = sm.tile([P, U], F32, tag="ctxp")
                    if NCH == 1:
                        nc.vector.tensor_copy(ctxp, parts[:, 0, :])
                    else:
                        h1 = sm.tile([P, U], F32, tag="ctx_h1")
                        nc.vector.tensor_add(h1, parts[:, 0, :],
                                             parts[:, 1, :])
                        for c in range(2, NCH):
                            h2 = sm.tile([P, U], F32, tag="ctx_h1")
                            nc.vector.tensor_add(h2, h1, parts[:, c, :])
                            h1 = h2
                        ctxp = h1

                    ctxn = sm.tile([P, U], F32, tag="ctxn")
                    nc.vector.tensor_scalar(
                        out=ctxn, in0=ctxp, scalar1=rs, scalar2=None,
                        op0=mybir.AluOpType.mult,
                    )

                    # GRU (h0 = 0): gates = ctx_aug.T @ [gk_z|gk_h; gb]
                    cT = tps.tile([U, P], F32, tag="tiny_ps")
                    nc.tensor.transpose(cT, ctxn, ident_sb)
                    caug = sm.tile([KA, P], F32, tag="caug")
                    nc.vector.memset(caug, 0.0)
                    nc.vector.memset(caug[0:1, :], 1.0)
                    nc.vector.tensor_copy(caug[32:48, :], cT)
                    gates = tps.tile([P, 2 * U], F32, tag="tiny_ps")
                    nc.tensor.matmul(gates, lhsT=caug, rhs=wg_sb,
                                     start=True, stop=True)
                    tz = sm.tile([P, U], F32, tag="tz")
                    nc.scalar.activation(tz, gates[:, 0:U],
                                         mybir.ActivationFunctionType.Tanh,
                                         scale=0.5)
                    th_g = sm.tile([P, U], F32, tag="th_g")
                    nc.scalar.activation(th_g, gates[:, U:2 * U],
                                         mybir.ActivationFunctionType.Tanh)
                    # hs = (tz - 1) * tanh(xh) = -2 * new_h
                    newh = sm.tile([P, U], F32, tag="newh")
                    nc.vector.scalar_tensor_tensor(
                        out=newh, in0=tz, scalar=1.0, in1=th_g,
                        op0=mybir.AluOpType.subtract,
                        op1=mybir.AluOpType.mult,
                    )

                    hT2 = tps.tile([U, P], F32, tag="tiny_ps")
                    nc.tensor.transpose(hT2, newh, ident_sb)
                    nc.vector.tensor_copy(haug[bt][32:48, :], hT2)
                    o_ps = tps.tile([P, O], F32, tag="tiny_ps")
                    nc.tensor.matmul(o_ps, lhsT=haug[bt], rhs=wd_sb,
                                     start=True, stop=True)
                    nc.vector.tensor_copy(
                        outacc[bt][:, s * O:(s + 1) * O], o_ps)

            for bt in range(NB):
                nc.sync.dma_start(
                    out=out[bt * P:(bt + 1) * P, :, :].rearrange(
                        "b s o -> b (s o)"),
                    in_=outacc[bt],
                )

    if legalize:
        _legalize_sync_waits(nc)
    return nc


def _pack_weights(w1, w2_k, w2_b, w3_k, gru_k, gru_b, dense_k, dense_b):
    U_ = w1.shape[0]
    w3 = np.asarray(w3_k, np.float32).reshape(U_)
    kron = np.kron(np.eye(P // U_, dtype=np.float16),
                   np.asarray(w1, np.float16))

    # augmented [48, n] weights: row 0 = bias, rows 32:48 = kernel,
    # rows 1:32 = zero. Device h-state is hs = -2*h, so the h-consuming
    # kernels (w2, dense) are scaled by -0.5.
    def aug(kern, bias):
        m = np.zeros((KA, kern.shape[1]), np.float32)
        m[0, :] = bias
        m[32:48, :] = kern
        return m

    wq = aug(np.asarray(w2_k, np.float32) * -0.5, np.asarray(w2_b, np.float32))
    gk = np.asarray(gru_k, np.float32)
    gb = np.asarray(gru_b, np.float32)
    wg = aug(np.hstack([gk[:, 0:U_], gk[:, 2 * U_:3 * U_]]),
             np.hstack([gb[0:U_], gb[2 * U_:3 * U_]]))
    wd = aug(np.asarray(dense_k, np.float32) * -0.5,
             np.asarray(dense_b, np.float32))
    return dict(ident16=np.eye(P, dtype=np.float16),
                w3ck=np.broadcast_to(w3.reshape(1, U_),
                                     (P, U_)).astype(np.float32).copy(),
                kronw1=kron, wq=wq, wg=wg, wd=wd,
                ident=np.eye(P, dtype=np.float32))


_PROGRAM_CACHE = {}


def kernel(num_inputs, enc_output, hidden, w1, w2_k, w2_b, w3_k, w3_b,
           gru_k, gru_rk, gru_b, dense_k, dense_b):
    from concourse.bass_utils import run_bass_kernel_spmd

    S = int(num_inputs)
    enc_output = np.asarray(enc_output, np.float32)
    hidden_np = np.asarray(hidden, np.float32)
    B, T, U_ = enc_output.shape
    B_c = B // N_CORES

    key = (B_c, T, S)
    if key not in _PROGRAM_CACHE:
        _PROGRAM_CACHE[key] = build_program(B_c, T, S)
    nc = _PROGRAM_CACHE[key]

    w = _pack_weights(w1, w2_k, w2_b, w3_k, gru_k, gru_b, dense_k, dense_b)

    in_maps = []
    for c in range(N_CORES):
        m = dict(w)
        m["enc"] = enc_output[c * B_c:(c + 1) * B_c]
        # device h-state convention is hs = -2*h
        m["hidden"] = hidden_np[c * B_c:(c + 1) * B_c] * np.float32(-2.0)
        in_maps.append(m)

    res = run_bass_kernel_spmd(nc, in_maps, core_ids=list(range(N_CORES)))
    outs = [res.results[c]["out"].reshape(B_c, S, O) for c in range(N_CORES)]
    return np.concatenate(outs, axis=0).astype(np.float32)



# revision 7
# speedup vs baseline: 1.6587x; 1.6587x over previous
"""Trainium2 Bass kernel for nn_Decoder (Bahdanau-attention decoder scan).

Contract: kernel(**inputs) takes FULL unsharded numpy inputs (keys as in
setup_inputs()) and returns the FULL [B, S, O] float32 output.

Sharding: pure data-parallel over batch B across 8 NeuronCores; weights
replicated; scan carry local per core.

Per-core algorithm (B_c = B/8 = 256, T = 2048, U = 16), v2:
  pre-pass (TileContext #1):
    enc fp32 -> fp16; encw1 = enc @ w1 on PE via a block-diag kron(I8, w1)
    stationary against DMA-xbar-transposed tiles; encw1 lands as two
    resident *u-major* fp16 SBUF tensors [128b, U, T] (contiguous per-u
    planes -> full-rate ACT reads); enc fp16 echo goes to DRAM scratch
    in u-major [b][u][t] layout for per-step streaming (contiguous 4KB
    per-partition DMA reads, packed DVE reads).
  scan (TileContext #2), per step s (bt-staggered emission so the ACT
  FIFO never waits on a GRU tail), per batch tile bt:
    q       = h_aug.T @ [w2_k; w2_b]                    (PE, tiny)
    th_u    = tanh(W[:, u, :] + q[:, u])                (ScalarE, fused
                                                         bias, contiguous)
    score   = sum_u w3[u] * th_u                        (DVE chain of
                                                         scalar_tensor_tensor
                                                         at 4x fp16 rate;
                                                         no PE, no LDWEIGHTS)
    e       = exp(score), sum_e via fused accum_out     (ScalarE; no max-sub
                                                         needed: |score|<~1.3)
    ctx_u   = sum_t e * enc_u / sum_e                   (DVE STT with
                                                         accum_out, per-u
                                                         contiguous planes)
    GRU with h0=0 degenerates:  new_h = 0.5*(1-tanh(xz/2))*tanh(xh)
    (sigmoid avoided => single ACT table set for the whole kernel)
    out_s   = new_h_aug.T @ [dense_k; dense_b]          (PE, tiny)

Two TileContexts: the pre-pass exit barrier resets Tile's vector clocks, so
scan-loop instructions don't inherit one sync-wait per pre-pass DMA HW queue
(walrus allows very few sync waits per datapath instruction).
Resident/state data lives in raw alloc_sbuf_tensor allocations that
survive across the contexts.
"""

import sys

import numpy as np

sys.path.insert(0, "/opt/trn_rl_repo")

import concourse.bass as bass  # noqa: E402
import concourse.tile as tile  # noqa: E402
from concourse import mybir  # noqa: E402

F32 = mybir.dt.float32
F16 = mybir.dt.float16

# Instructions that never carry lowered sync waits / have no events field.
_MULTIWAIT_OK = {"InstUnconditionalBranch", "InstCall",
                 "InstRegisterMove", "InstRegisterAlu"}


def _legalize_sync_waits(nc, max_waits=1):
    """Walrus' codegen allows very few sync-wait commands per datapath
    instruction (matmul LW and TensorScalarPtr fail at 2). Engine queues
    are strict FIFO, so we can splice single-wait NOPs in front of any
    instruction that carries more than `max_waits` waits and leave only
    the last wait on the instruction itself."""
    k = 0
    for func in nc.m.functions:
        for bb in func.blocks:
            il = bb.instructions
            i = 0
            while i < len(il):
                ins = il[i]
                si = ins.sync_info
                if (type(ins).__name__ not in _MULTIWAIT_OK
                        and si is not None and si.on_wait
                        and len(si.on_wait) > max_waits):
                    waits = list(si.on_wait)
                    for w in waits[:-max_waits]:
                        nop = mybir.InstNoOp(name=f"syncsplit-{k}",
                                             ins=[], outs=[])
                        k += 1
                        nop.engine = ins.engine
                        nop.sync_info = mybir.SyncInfo(on_wait=[w],
                                                       on_update=[])
                        il.insert(i, nop)
                        i += 1
                    ins.sync_info = mybir.SyncInfo(
                        on_wait=waits[-max_waits:],
                        on_update=list(si.on_update or []))
                i += 1
    return k


N_CORES = 8
U = 16
O = 8
P = 128  # partitions
KA = 48  # augmented contraction: row 0 = bias, rows 32:48 = data


def build_program(B_c, T, S, legalize=True):
    """Build the single-core bass program (same program runs SPMD per core)."""
    assert B_c % P == 0 and T % P == 0
    NB = B_c // P
    TLO = 8
    NTHI = T // TLO
    CT = min(128, T)  # t-chunk for the fp32 load/convert stage
    AXROWS = 2048
    assert (NTHI * B_c) % AXROWS == 0
    assert (T * U) % AXROWS == 0

    nc = bass.Bass("TRN2", target_bir_lowering=False)

    enc = nc.dram_tensor("enc", [B_c, T, U], F32, kind="ExternalInput").ap()
    hidden = nc.dram_tensor("hidden", [B_c, U], F32, kind="ExternalInput").ap()
    w3ck = nc.dram_tensor("w3ck", [P, U], F32, kind="ExternalInput").ap()
    kronw1 = nc.dram_tensor("kronw1", [P, P], F16, kind="ExternalInput").ap()
    wq = nc.dram_tensor("wq", [KA, U], F32, kind="ExternalInput").ap()
    wg = nc.dram_tensor("wg", [KA, 2 * U], F32, kind="ExternalInput").ap()
    wd = nc.dram_tensor("wd", [KA, O], F32, kind="ExternalInput").ap()
    ident = nc.dram_tensor("ident", [P, P], F32, kind="ExternalInput").ap()
    out = nc.dram_tensor("out", [B_c, S, O], F32, kind="ExternalOutput").ap()

    # DRAM scratch
    tmp = nc.dram_tensor("tmp_bm", [NTHI, B_c, TLO * U], F16).ap()
    w1x = nc.dram_tensor("w1x", [U * T, B_c], F16).ap()  # rows = (u, t)
    encsc = nc.dram_tensor("encsc", [B_c, U, T], F16).ap()  # u-major

    # raw SBUF residents (survive across both TileContexts)
    w1res = [nc.alloc_sbuf_tensor(f"w1res{bt}", [P, U, T], F16).ap()
             for bt in range(NB)]
    ident_sb = nc.alloc_sbuf_tensor("ident_r", [P, P], F32).ap()
    w3_sb = nc.alloc_sbuf_tensor("w3_r", [P, U], F32).ap()
    wq_sb = nc.alloc_sbuf_tensor("wq_r", [KA, U], F32).ap()
    wg_sb = nc.alloc_sbuf_tensor("wg_r", [KA, 2 * U], F32).ap()
    wd_sb = nc.alloc_sbuf_tensor("wd_r", [KA, O], F32).ap()
    haug = [nc.alloc_sbuf_tensor(f"haug{bt}", [KA, P], F32).ap()
            for bt in range(NB)]

    # ================= TileContext 1: pre-pass =================
    with tile.TileContext(nc) as tc:
        with tc.tile_pool(name="pp_psum", bufs=2, space="PSUM") as pp_psum, \
             tc.tile_pool(name="pp_sbuf", bufs=2) as pp:

            # small weights into residents
            nc.sync.dma_start(out=ident_sb, in_=ident)
            nc.sync.dma_start(out=w3_sb, in_=w3ck)
            nc.sync.dma_start(out=wq_sb, in_=wq)
            nc.sync.dma_start(out=wg_sb, in_=wg)
            nc.sync.dma_start(out=wd_sb, in_=wd)

            kron_sb = pp.tile([P, P], F16, tag="kron")
            nc.sync.dma_start(out=kron_sb, in_=kronw1)

            # h_aug init from `hidden` (host pre-scales hidden by -2)
            for bt in range(NB):
                h0 = pp.tile([P, U], F32, tag="h0")
                nc.sync.dma_start(out=h0, in_=hidden[bt * P:(bt + 1) * P, :])
                hT = pp_psum.tile([U, P], F32, tag="hT")
                nc.tensor.transpose(hT, h0, ident_sb)
                nc.vector.memset(haug[bt], 0.0)
                nc.vector.memset(haug[bt][0:1, :], 1.0)
                nc.vector.tensor_copy(haug[bt][32:48, :], hT)

            # phase 1: fp32 load -> fp16 -> tmp [thi][b][tlo*u]
            # (+ u-major encsc echo via on-chip strided transpose-copy)
            for bt in range(NB):
                for c in range(T // CT):
                    ld = pp.tile([P, CT, U], F32, tag="pp_ld")
                    nc.sync.dma_start(
                        out=ld,
                        in_=enc[bt * P:(bt + 1) * P, c * CT:(c + 1) * CT, :],
                    )
                    cv = pp.tile([P, CT, U], F16, tag="pp_cv")
                    nc.vector.tensor_copy(cv, ld)
                    cvt = pp.tile([P, U, CT], F16, tag="pp_cvt")
                    nc.vector.tensor_copy(
                        cvt, cv.rearrange("b t u -> b u t"))
                    nc.scalar.dma_start(
                        out=encsc[bt * P:(bt + 1) * P, :,
                                  c * CT:(c + 1) * CT],
                        in_=cvt,
                    )
                    tv = tmp.rearrange("thi b i -> b thi i")
                    nthi = CT // TLO
                    nc.sync.dma_start(
                        out=tv[bt * P:(bt + 1) * P,
                               c * nthi:(c + 1) * nthi, :],
                        in_=cv.rearrange("b (thi tlo) u -> b thi (tlo u)",
                                         tlo=TLO),
                    )

            # phase 2: A-xbar transpose + blockdiag matmul -> w1x [(u t), b]
            # (u-major row order so phase 3 lands contiguous u-planes).
            # evbig partitions are (tlo, v) but w1x rows are (v, thi, tlo),
            # which is not AP-expressible in one go -- write one DMA per
            # tlo slice (16 v-partitions each), alternating trigger queues.
            tmp_rows = tmp.rearrange("thi b i -> (thi b) i")
            w1_v = w1x.rearrange("(v thi tlo) b -> tlo v thi b",
                                 v=U, tlo=TLO)
            nax = (NTHI * B_c) // AXROWS
            thi_per_ax = AXROWS // B_c
            for r in range(nax):
                ax = pp.tile([P, AXROWS], F16, tag="pp_ax")
                nc.sync.dma_start(
                    out=ax,
                    in_=tmp_rows[r * AXROWS:(r + 1) * AXROWS, :],
                    transpose=True,
                )
                evbig = pp.tile([P, AXROWS], F16, tag="pp_ev")
                nmm = AXROWS // 512
                for n in range(nmm):
                    ps = pp_psum.tile([P, 512], F32, tag="pp_bd")
                    nc.tensor.matmul(ps, lhsT=kron_sb,
                                     rhs=ax[:, n * 512:(n + 1) * 512],
                                     start=True, stop=True)
                    if n % 2 == 0:
                        nc.vector.tensor_copy(
                            evbig[:, n * 512:(n + 1) * 512], ps)
                    else:
                        nc.scalar.copy(evbig[:, n * 512:(n + 1) * 512], ps)
                ts = slice(r * thi_per_ax, (r + 1) * thi_per_ax)
                ev3 = evbig.rearrange("p (thi b) -> p thi b",
                                      thi=thi_per_ax)
                for c in range(TLO):
                    eng = nc.sync if c % 2 == 0 else nc.scalar
                    eng.dma_start(
                        out=w1_v[c, :, ts, :],
                        in_=ev3[c * U:(c + 1) * U, :, :],
                    )

            # phase 3: B-xbar -> resident u-major encw1 tensors
            for bt in range(NB):
                plv = w1res[bt].rearrange("b u t -> b (u t)")
                for r in range((T * U) // AXROWS):
                    nc.sync.dma_start(
                        out=plv[:, r * AXROWS:(r + 1) * AXROWS],
                        in_=w1x[r * AXROWS:(r + 1) * AXROWS,
                                bt * P:(bt + 1) * P],
                        transpose=True,
                    )

    TH = T // 2  # stream half-plane length
    UG = 4       # u-planes per streamed group

    # ================= TileContext 2: the decoder scan =================
    with tile.TileContext(nc) as tc2:
        with tc2.tile_pool(name="tiny_psum", bufs=3, space="PSUM") as tps, \
             tc2.tile_pool(name="planes", bufs=3) as planes, \
             tc2.tile_pool(name="scoreb", bufs=2) as scoreb, \
             tc2.tile_pool(name="stream", bufs=4) as stream, \
             tc2.tile_pool(name="sm", bufs=2) as sm, \
             tc2.tile_pool(name="junk", bufs=2) as junkp, \
             tc2.tile_pool(name="outp", bufs=1) as outp:

            outacc = []
            for bt in range(NB):
                oa = outp.tile([P, S * O], F32, tag=f"outacc{bt}",
                               name=f"outacc{bt}")
                outacc.append(oa)

            def head_phase(s, bt):
                """q -> 16x tanh -> DVE w3-weighted score chain -> exp.
                Returns (e_sb, rs)."""
                q_ps = tps.tile([P, U], F32, tag="tiny_ps")
                nc.tensor.matmul(q_ps, lhsT=haug[bt], rhs=wq_sb,
                                 start=True, stop=True)
                q_sb = sm.tile([P, U], F32, tag="q_sb")
                nc.vector.tensor_copy(q_sb, q_ps)

                sc_prev = None
                for u in range(U):
                    th = planes.tile([P, T], F16, tag="tanh_plane")
                    nc.scalar.activation(
                        th, w1res[bt][:, u, :],
                        mybir.ActivationFunctionType.Tanh,
                        bias=q_sb[:, u:u + 1], scale=1.0,
                    )
                    sc = scoreb.tile([P, T], F16, tag="score")
                    if u == 0:
                        nc.vector.tensor_scalar(
                            out=sc, in0=th, scalar1=w3_sb[:, 0:1],
                            scalar2=None, op0=mybir.AluOpType.mult,
                        )
                    else:
                        nc.vector.scalar_tensor_tensor(
                            out=sc, in0=th, scalar=w3_sb[:, u:u + 1],
                            in1=sc_prev,
                            op0=mybir.AluOpType.mult,
                            op1=mybir.AluOpType.add,
                        )
                    sc_prev = sc

                e_sb = sm.tile([P, T], F16, tag="e_sb")
                sum_e = sm.tile([P, 1], F32, tag="sum_e")
                nc.scalar.activation(
                    e_sb, sc_prev, mybir.ActivationFunctionType.Exp,
                    accum_out=sum_e,
                )
                rs = sm.tile([P, 1], F32, tag="rs")
                nc.vector.reciprocal(rs, sum_e)
                return e_sb, rs

            def tail_phase(s, bt, e_sb, rs):
                """stream enc u-groups + ctx reduce -> GRU -> output row.
                The enc stream is kicked here (16 MB/s/bt of DMA hides
                under the other bt's ~33us ACT tanh block)."""
                bsl = slice(bt * P, (bt + 1) * P)
                cparts = sm.tile([P, 2, U], F32, tag="cparts")
                for h in range(2):
                    tsl = slice(h * TH, (h + 1) * TH)
                    for g in range(U // UG):
                        ec = stream.tile([P, UG, TH], F16, tag="ec")
                        nc.sync.dma_start(
                            out=ec,
                            in_=encsc[bsl, g * UG:(g + 1) * UG, tsl],
                        )
                        for j in range(UG):
                            junk = junkp.tile([P, TH], F16, tag="junk")
                            nc.vector.scalar_tensor_tensor(
                                out=junk, in0=e_sb[:, tsl], scalar=1.0,
                                in1=ec[:, j, :],
                                op0=mybir.AluOpType.mult,
                                op1=mybir.AluOpType.mult,
                                accum_out=cparts[:, h, g * UG + j:
                                                 g * UG + j + 1],
                            )

                ctxp = sm.tile([P, U], F32, tag="ctxp")
                nc.vector.tensor_add(ctxp, cparts[:, 0, :], cparts[:, 1, :])
                ctxn = sm.tile([P, U], F32, tag="ctxn")
                nc.vector.tensor_scalar(
                    out=ctxn, in0=ctxp, scalar1=rs, scalar2=None,
                    op0=mybir.AluOpType.mult,
                )

                # GRU (h0 = 0): gates = ctx_aug.T @ [gk_z|gk_h; gb]
                cT = tps.tile([U, P], F32, tag="tiny_ps")
                nc.tensor.transpose(cT, ctxn, ident_sb)
                caug = sm.tile([KA, P], F32, tag="caug")
                nc.vector.memset(caug, 0.0)
                nc.vector.memset(caug[0:1, :], 1.0)
                nc.vector.tensor_copy(caug[32:48, :], cT)
                gates = tps.tile([P, 2 * U], F32, tag="tiny_ps")
                nc.tensor.matmul(gates, lhsT=caug, rhs=wg_sb,
                                 start=True, stop=True)
                tz = sm.tile([P, U], F32, tag="tz")
                nc.scalar.activation(tz, gates[:, 0:U],
                                     mybir.ActivationFunctionType.Tanh,
                                     scale=0.5)
                th_g = sm.tile([P, U], F32, tag="th_g")
                nc.scalar.activation(th_g, gates[:, U:2 * U],
                                     mybir.ActivationFunctionType.Tanh)
                # hs = (tz - 1) * tanh(xh) = -2 * new_h
                newh = sm.tile([P, U], F32, tag="newh")
                nc.vector.scalar_tensor_tensor(
                    out=newh, in0=tz, scalar=1.0, in1=th_g,
                    op0=mybir.AluOpType.subtract,
                    op1=mybir.AluOpType.mult,
                )

                hT2 = tps.tile([U, P], F32, tag="tiny_ps")
                nc.tensor.transpose(hT2, newh, ident_sb)
                nc.vector.tensor_copy(haug[bt][32:48, :], hT2)
                o_ps = tps.tile([P, O], F32, tag="tiny_ps")
                nc.tensor.matmul(o_ps, lhsT=haug[bt], rhs=wd_sb,
                                 start=True, stop=True)
                nc.vector.tensor_copy(
                    outacc[bt][:, s * O:(s + 1) * O], o_ps)

            # bt-staggered emission: each bt's tail (stream DMA + ctx
            # reduce + GRU) is emitted under the OTHER bt's ACT tanh
            # block, so the ACT FIFO never queues a GRU tanh before a
            # ready tanh block and the enc stream DMA hides fully.
            assert NB == 2
            pend = {}  # bt -> (s, e_sb, rs)
            for s in range(S):
                for bt in range(NB):
                    e_sb, rs = head_phase(s, bt)
                    other = 1 - bt
                    if other in pend:
                        ps, pe, prs = pend.pop(other)
                        tail_phase(ps, other, pe, prs)
                    pend[bt] = (s, e_sb, rs)
            for bt in (0, 1):
                if bt in pend:
                    ps, pe, prs = pend.pop(bt)
                    tail_phase(ps, bt, pe, prs)

            for bt in range(NB):
                nc.sync.dma_start(
                    out=out[bt * P:(bt + 1) * P, :, :].rearrange(
                        "b s o -> b (s o)"),
                    in_=outacc[bt],
                )

    if legalize:
        _legalize_sync_waits(nc)
    return nc


def _pack_weights(w1, w2_k, w2_b, w3_k, gru_k, gru_b, dense_k, dense_b):
    U_ = w1.shape[0]
    w3 = np.asarray(w3_k, np.float32).reshape(U_)
    kron = np.kron(np.eye(P // U_, dtype=np.float16),
                   np.asarray(w1, np.float16))

    # augmented [48, n] weights: row 0 = bias, rows 32:48 = kernel,
    # rows 1:32 = zero. Device h-state is hs = -2*h, so the h-consuming
    # kernels (w2, dense) are scaled by -0.5.
    def aug(kern, bias):
        m = np.zeros((KA, kern.shape[1]), np.float32)
        m[0, :] = bias
        m[32:48, :] = kern
        return m

    wq = aug(np.asarray(w2_k, np.float32) * -0.5, np.asarray(w2_b, np.float32))
    gk = np.asarray(gru_k, np.float32)
    gb = np.asarray(gru_b, np.float32)
    wg = aug(np.hstack([gk[:, 0:U_], gk[:, 2 * U_:3 * U_]]),
             np.hstack([gb[0:U_], gb[2 * U_:3 * U_]]))
    wd = aug(np.asarray(dense_k, np.float32) * -0.5,
             np.asarray(dense_b, np.float32))
    return dict(w3ck=np.broadcast_to(w3.reshape(1, U_),
                                     (P, U_)).astype(np.float32).copy(),
                kronw1=kron, wq=wq, wg=wg, wd=wd,
                ident=np.eye(P, dtype=np.float32))


_PROGRAM_CACHE = {}


def kernel(num_inputs, enc_output, hidden, w1, w2_k, w2_b, w3_k, w3_b,
           gru_k, gru_rk, gru_b, dense_k, dense_b):
    from concourse.bass_utils import run_bass_kernel_spmd

    S = int(num_inputs)
    enc_output = np.asarray(enc_output, np.float32)
    hidden_np = np.asarray(hidden, np.float32)
    B, T, U_ = enc_output.shape
    B_c = B // N_CORES

    key = (B_c, T, S)
    if key not in _PROGRAM_CACHE:
        _PROGRAM_CACHE[key] = build_program(B_c, T, S)
    nc = _PROGRAM_CACHE[key]

    w = _pack_weights(w1, w2_k, w2_b, w3_k, gru_k, gru_b, dense_k, dense_b)

    in_maps = []
    for c in range(N_CORES):
        m = dict(w)
        m["enc"] = enc_output[c * B_c:(c + 1) * B_c]
        # device h-state convention is hs = -2*h
        m["hidden"] = hidden_np[c * B_c:(c + 1) * B_c] * np.float32(-2.0)
        in_maps.append(m)

    res = run_bass_kernel_spmd(nc, in_maps, core_ids=list(range(N_CORES)))
    outs = [res.results[c]["out"].reshape(B_c, S, O) for c in range(N_CORES)]
    return np.concatenate(outs, axis=0).astype(np.float32)


# revision 19
# speedup vs baseline: 1.6822x; 1.0142x over previous
"""Trainium2 Bass kernel for nn_Decoder (Bahdanau-attention decoder scan).

Contract: kernel(**inputs) takes FULL unsharded numpy inputs (keys as in
setup_inputs()) and returns the FULL [B, S, O] float32 output.

Sharding: pure data-parallel over batch B across 8 NeuronCores; weights
replicated; scan carry local per core.

Per-core algorithm (B_c = B/8 = 256, T = 2048, U = 16), v2:
  pre-pass (TileContext #1):
    enc fp32 -> fp16; encw1 = enc @ w1 on PE via a block-diag kron(I8, w1)
    stationary against DMA-xbar-transposed tiles; encw1 lands as two
    resident *u-major* fp16 SBUF tensors [128b, U, T] (contiguous per-u
    planes -> full-rate ACT reads); enc fp16 echo goes to DRAM scratch
    in u-major [b][u][t] layout for per-step streaming (contiguous 4KB
    per-partition DMA reads, packed DVE reads).
  scan (TileContext #2), per step s (bt-staggered emission so the ACT
  FIFO never waits on a GRU tail), per batch tile bt:
    q       = h_aug.T @ [w2_k; w2_b]                    (PE, tiny)
    th_u    = tanh(W[:, u, :] + q[:, u])                (ScalarE, fused
                                                         bias, contiguous)
    score   = sum_u w3[u] * th_u                        (DVE chain of
                                                         scalar_tensor_tensor
                                                         at 4x fp16 rate;
                                                         no PE, no LDWEIGHTS)
    e       = exp(score), sum_e via fused accum_out     (ScalarE; no max-sub
                                                         needed: |score|<~1.3)
    ctx_u   = sum_t e * enc_u / sum_e                   (DVE STT with
                                                         accum_out, per-u
                                                         contiguous planes)
    GRU with h0=0 degenerates:  new_h = 0.5*(1-tanh(xz/2))*tanh(xh)
    (sigmoid avoided => single ACT table set for the whole kernel)
    out_s   = new_h_aug.T @ [dense_k; dense_b]          (PE, tiny)

Two TileContexts: the pre-pass exit barrier resets Tile's vector clocks, so
scan-loop instructions don't inherit one sync-wait per pre-pass DMA HW queue
(walrus allows very few sync waits per datapath instruction).
Resident/state data lives in raw alloc_sbuf_tensor allocations that
survive across the contexts.
"""

import sys

import numpy as np

sys.path.insert(0, "/opt/trn_rl_repo")

import concourse.bass as bass  # noqa: E402
import concourse.tile as tile  # noqa: E402
from concourse import mybir  # noqa: E402

F32 = mybir.dt.float32
F16 = mybir.dt.float16

# Instructions that never carry lowered sync waits / have no events field.
_MULTIWAIT_OK = {"InstUnconditionalBranch", "InstCall",
                 "InstRegisterMove", "InstRegisterAlu"}


def _legalize_sync_waits(nc, max_waits=1):
    """Walrus' codegen allows very few sync-wait commands per datapath
    instruction (matmul LW and TensorScalarPtr fail at 2). Engine queues
    are strict FIFO, so we can splice single-wait NOPs in front of any
    instruction that carries more than `max_waits` waits and leave only
    the last wait on the instruction itself."""
    k = 0
    for func in nc.m.functions:
        for bb in func.blocks:
            il = bb.instructions
            i = 0
            while i < len(il):
                ins = il[i]
                si = ins.sync_info
                if (type(ins).__name__ not in _MULTIWAIT_OK
                        and si is not None and si.on_wait
                        and len(si.on_wait) > max_waits):
                    waits = list(si.on_wait)
                    for w in waits[:-max_waits]:
                        nop = mybir.InstNoOp(name=f"syncsplit-{k}",
                                             ins=[], outs=[])
                        k += 1
                        nop.engine = ins.engine
                        nop.sync_info = mybir.SyncInfo(on_wait=[w],
                                                       on_update=[])
                        il.insert(i, nop)
                        i += 1
                    ins.sync_info = mybir.SyncInfo(
                        on_wait=waits[-max_waits:],
                        on_update=list(si.on_update or []))
                i += 1
    return k


N_CORES = 8
U = 16
O = 8
P = 128  # partitions
KA = 48  # augmented contraction: row 0 = bias, rows 32:48 = data


def build_program(B_c, T, S, legalize=True):
    """Build the single-core bass program (same program runs SPMD per core)."""
    assert B_c % P == 0 and T % P == 0
    NB = B_c // P
    TLO = 8
    NTHI = T // TLO
    CT = min(128, T)  # t-chunk for the fp32 load/convert stage
    AXROWS = 2048
    assert (NTHI * B_c) % AXROWS == 0
    assert (T * U) % AXROWS == 0

    nc = bass.Bass("TRN2", target_bir_lowering=False)

    enc = nc.dram_tensor("enc", [B_c, T, U], F32, kind="ExternalInput").ap()
    hidden = nc.dram_tensor("hidden", [B_c, U], F32, kind="ExternalInput").ap()
    w3ck = nc.dram_tensor("w3ck", [P, U], F32, kind="ExternalInput").ap()
    kronw1 = nc.dram_tensor("kronw1", [P, P], F16, kind="ExternalInput").ap()
    wq = nc.dram_tensor("wq", [KA, U], F32, kind="ExternalInput").ap()
    wg = nc.dram_tensor("wg", [KA, 2 * U], F32, kind="ExternalInput").ap()
    wd = nc.dram_tensor("wd", [KA, O], F32, kind="ExternalInput").ap()
    ident = nc.dram_tensor("ident", [P, P], F32, kind="ExternalInput").ap()
    out = nc.dram_tensor("out", [B_c, S, O], F32, kind="ExternalOutput").ap()

    # DRAM scratch
    tmp = nc.dram_tensor("tmp_bm", [NTHI, B_c, TLO * U], F16).ap()
    w1x = nc.dram_tensor("w1x", [U * T, B_c], F16).ap()  # rows = (u, t)
    encsc = nc.dram_tensor("encsc", [U, B_c, T], F16).ap()  # xbar source
    encT = nc.dram_tensor("encT", [T, U, B_c], F16).ap()  # t-partition form

    # raw SBUF residents (survive across both TileContexts)
    w1res = [nc.alloc_sbuf_tensor(f"w1res{bt}", [P, U, T], F16).ap()
             for bt in range(NB)]
    ident_sb = nc.alloc_sbuf_tensor("ident_r", [P, P], F32).ap()
    idm16 = nc.alloc_sbuf_tensor("idm16_r", [P, P], F16).ap()
    w3_sb = nc.alloc_sbuf_tensor("w3_r", [P, U], F32).ap()
    wq_sb = nc.alloc_sbuf_tensor("wq_r", [KA, U], F32).ap()
    wg_sb = nc.alloc_sbuf_tensor("wg_r", [KA, 2 * U], F32).ap()
    wd_sb = nc.alloc_sbuf_tensor("wd_r", [KA, O], F32).ap()
    haug = [nc.alloc_sbuf_tensor(f"haug{bt}", [KA, P], F32).ap()
            for bt in range(NB)]

    # ================= TileContext 1: pre-pass =================
    with tile.TileContext(nc) as tc:
        with tc.tile_pool(name="pp_psum", bufs=2, space="PSUM") as pp_psum, \
             tc.tile_pool(name="pp_sbuf", bufs=2) as pp:

            # small weights into residents
            nc.sync.dma_start(out=ident_sb, in_=ident)
            nc.vector.tensor_copy(idm16, ident_sb)
            nc.sync.dma_start(out=w3_sb, in_=w3ck)
            nc.sync.dma_start(out=wq_sb, in_=wq)
            nc.sync.dma_start(out=wg_sb, in_=wg)
            nc.sync.dma_start(out=wd_sb, in_=wd)

            kron_sb = pp.tile([P, P], F16, tag="kron")
            nc.sync.dma_start(out=kron_sb, in_=kronw1)

            # h_aug init from `hidden` (host pre-scales hidden by -2)
            for bt in range(NB):
                h0 = pp.tile([P, U], F32, tag="h0")
                nc.sync.dma_start(out=h0, in_=hidden[bt * P:(bt + 1) * P, :])
                hT = pp_psum.tile([U, P], F32, tag="hT")
                nc.tensor.transpose(hT, h0, ident_sb)
                nc.vector.memset(haug[bt], 0.0)
                nc.vector.memset(haug[bt][0:1, :], 1.0)
                nc.vector.tensor_copy(haug[bt][32:48, :], hT)

            # phase 1: fp32 load -> fp16 -> tmp [thi][b][tlo*u]
            # (+ u-major encsc echo via on-chip strided transpose-copy)
            for bt in range(NB):
                for c in range(T // CT):
                    ld = pp.tile([P, CT, U], F32, tag="pp_ld")
                    nc.sync.dma_start(
                        out=ld,
                        in_=enc[bt * P:(bt + 1) * P, c * CT:(c + 1) * CT, :],
                    )
                    cv = pp.tile([P, CT, U], F16, tag="pp_cv")
                    nc.vector.tensor_copy(cv, ld)
                    cvt = pp.tile([P, U, CT], F16, tag="pp_cvt")
                    nc.vector.tensor_copy(
                        cvt, cv.rearrange("b t u -> b u t"))
                    encsc_b = encsc.rearrange("u b t -> b u t")
                    nc.scalar.dma_start(
                        out=encsc_b[bt * P:(bt + 1) * P, :,
                                    c * CT:(c + 1) * CT],
                        in_=cvt,
                    )
                    tv = tmp.rearrange("thi b i -> b thi i")
                    nthi = CT // TLO
                    nc.sync.dma_start(
                        out=tv[bt * P:(bt + 1) * P,
                               c * nthi:(c + 1) * nthi, :],
                        in_=cv.rearrange("b (thi tlo) u -> b thi (tlo u)",
                                         tlo=TLO),
                    )

            # phase 2: A-xbar transpose + blockdiag matmul -> w1x [(u t), b]
            # (u-major row order so phase 3 lands contiguous u-planes).
            # evbig partitions are (tlo, v) but w1x rows are (v, thi, tlo),
            # which is not AP-expressible in one go -- write one DMA per
            # tlo slice (16 v-partitions each), alternating trigger queues.
            tmp_rows = tmp.rearrange("thi b i -> (thi b) i")
            w1_v = w1x.rearrange("(v thi tlo) b -> tlo v thi b",
                                 v=U, tlo=TLO)
            nax = (NTHI * B_c) // AXROWS
            thi_per_ax = AXROWS // B_c
            for r in range(nax):
                ax = pp.tile([P, AXROWS], F16, tag="pp_ax")
                nc.sync.dma_start(
                    out=ax,
                    in_=tmp_rows[r * AXROWS:(r + 1) * AXROWS, :],
                    transpose=True,
                )
                evbig = pp.tile([P, AXROWS], F16, tag="pp_ev")
                nmm = AXROWS // 512
                for n in range(nmm):
                    ps = pp_psum.tile([P, 512], F32, tag="pp_bd")
                    nc.tensor.matmul(ps, lhsT=kron_sb,
                                     rhs=ax[:, n * 512:(n + 1) * 512],
                                     start=True, stop=True)
                    if n % 2 == 0:
                        nc.vector.tensor_copy(
                            evbig[:, n * 512:(n + 1) * 512], ps)
                    else:
                        nc.scalar.copy(evbig[:, n * 512:(n + 1) * 512], ps)
                ts = slice(r * thi_per_ax, (r + 1) * thi_per_ax)
                ev3 = evbig.rearrange("p (thi b) -> p thi b",
                                      thi=thi_per_ax)
                for c in range(TLO):
                    eng = nc.sync if c % 2 == 0 else nc.scalar
                    eng.dma_start(
                        out=w1_v[c, :, ts, :],
                        in_=ev3[c * U:(c + 1) * U, :, :],
                    )

            # phase 1.5: second xbar pass encsc [(u b), t] -> encT [t, (u b)]
            # (gives the scan a t-partition form of enc so the ctx reduce
            # can run on PE as a transposed-e matmul). On the ACT trigger
            # queue to keep it off the SP xbar queue's critical path.
            encsc_rows = encsc.rearrange("u b t -> (u b) t")
            encT_rows = encT.rearrange("t u b -> t (u b)")
            for j in range(T // P):
                tt = pp.tile([P, U * B_c], F16, tag="pp_tt")
                nc.scalar.dma_start(
                    out=tt,
                    in_=encsc_rows[:, j * P:(j + 1) * P],
                    transpose=True,
                )
                nc.scalar.dma_start(
                    out=encT_rows[j * P:(j + 1) * P, :],
                    in_=tt,
                )

            # phase 3: B-xbar -> resident u-major encw1 tensors
            for bt in range(NB):
                plv = w1res[bt].rearrange("b u t -> b (u t)")
                for r in range((T * U) // AXROWS):
                    nc.sync.dma_start(
                        out=plv[:, r * AXROWS:(r + 1) * AXROWS],
                        in_=w1x[r * AXROWS:(r + 1) * AXROWS,
                                bt * P:(bt + 1) * P],
                        transpose=True,
                    )

    TH = T // 2   # score-chain half length (DVE half / GpSimd half)
    NC_ = T // P  # t-chunks for the ctx matmul

    # ================= TileContext 2: the decoder scan =================
    with tile.TileContext(nc) as tc2:
        with tc2.tile_pool(name="ctx_psum", bufs=1, space="PSUM") as cps, \
             tc2.tile_pool(name="tr_psum", bufs=2, space="PSUM") as trps, \
             tc2.tile_pool(name="tiny_psum", bufs=2, space="PSUM") as tps, \
             tc2.tile_pool(name="planes", bufs=3) as planes, \
             tc2.tile_pool(name="scoreb", bufs=2) as scoreb, \
             tc2.tile_pool(name="stream", bufs=4) as stream, \
             tc2.tile_pool(name="etp", bufs=4) as etp, \
             tc2.tile_pool(name="sm", bufs=2) as sm, \
             tc2.tile_pool(name="junk", bufs=2) as junkp, \
             tc2.tile_pool(name="outp", bufs=1) as outp:

            outacc = []
            for bt in range(NB):
                oa = outp.tile([P, S * O], F32, tag=f"outacc{bt}",
                               name=f"outacc{bt}")
                outacc.append(oa)

            def head_phase(s, bt):
                """q -> 16x tanh -> w3-weighted score chains (DVE on the
                low t-half, GpSimd on the high t-half, both chasing ACT)
                -> exp per half. Returns (e0, e1, rs)."""
                q_ps = tps.tile([P, U], F32, tag="tiny_ps")
                nc.tensor.matmul(q_ps, lhsT=haug[bt], rhs=wq_sb,
                                 start=True, stop=True)
                q_sb = sm.tile([P, U], F32, tag="q_sb")
                nc.vector.tensor_copy(q_sb, q_ps)

                sc_prev = None
                for u in range(U):
                    th = planes.tile([P, T], F16, tag="tanh_plane")
                    nc.scalar.activation(
                        th, w1res[bt][:, u, :],
                        mybir.ActivationFunctionType.Tanh,
                        bias=q_sb[:, u:u + 1], scale=1.0,
                    )
                    sc = scoreb.tile([P, T], F16, tag="score")
                    if u == 0:
                        nc.vector.tensor_scalar(
                            out=sc, in0=th, scalar1=w3_sb[:, 0:1],
                            scalar2=None, op0=mybir.AluOpType.mult,
                        )
                    elif u == 1:
                        # probe: TS + plain TENSOR_TENSOR add, to measure
                        # whether InstTensorTensor hits 2x on HW (the STT
                        # form measured 1x)
                        tsx = planes.tile([P, T], F16, tag="ts_probe")
                        nc.vector.tensor_scalar(
                            out=tsx, in0=th, scalar1=w3_sb[:, u:u + 1],
                            scalar2=None, op0=mybir.AluOpType.mult,
                        )
                        nc.vector.tensor_add(sc, tsx, sc_prev)
                    else:
                        nc.vector.scalar_tensor_tensor(
                            out=sc, in0=th, scalar=w3_sb[:, u:u + 1],
                            in1=sc_prev,
                            op0=mybir.AluOpType.mult,
                            op1=mybir.AluOpType.add,
                        )
                    sc_prev = sc

                e_sb = sm.tile([P, T], F16, tag="e_sb")
                sum_e = sm.tile([P, 1], F32, tag="sum_e")
                nc.scalar.activation(
                    e_sb, sc_prev, mybir.ActivationFunctionType.Exp,
                    accum_out=sum_e,
                )
                rs = sm.tile([P, 1], F32, tag="rs")
                nc.vector.reciprocal(rs, sum_e)
                return e_sb, rs

            def tail_phase(s, bt, e_sb, rs):
                """ctx reduce on PE: stream encT t-chunks, transpose e
                per chunk, accumulate eT.T @ encT into PSUM [b, (u, b')],
                extract the b'=b diagonal per u via tiny masked STTs."""
                bsl = slice(bt * P, (bt + 1) * P)
                ctx_ps = cps.tile([P, U * P], F32, tag="ctx_ps")
                for c in range(NC_):
                    ec = stream.tile([P, U, P], F16, tag="ec")
                    nc.sync.dma_start(
                        out=ec, in_=encT[c * P:(c + 1) * P, :, bsl],
                    )
                    psT = trps.tile([P, P], F16, tag="psT")
                    nc.tensor.transpose(
                        psT, e_sb[:, c * P:(c + 1) * P], idm16)
                    eTt = etp.tile([P, P], F16, tag="eTt")
                    nc.vector.tensor_copy(eTt, psT)
                    rhs = ec.rearrange("t u b -> t (u b)")
                    for q in range(U * P // 512):
                        nc.tensor.matmul(
                            ctx_ps[:, q * 512:(q + 1) * 512],
                            lhsT=eTt, rhs=rhs[:, q * 512:(q + 1) * 512],
                            start=(c == 0), stop=(c == NC_ - 1),
                        )

                ctxp = sm.tile([P, U], F32, tag="ctxp")
                for u in range(U):
                    junk = junkp.tile([P, P], F16, tag="junk")
                    nc.vector.scalar_tensor_tensor(
                        out=junk, in0=ctx_ps[:, u * P:(u + 1) * P],
                        scalar=1.0, in1=idm16,
                        op0=mybir.AluOpType.mult,
                        op1=mybir.AluOpType.mult,
                        accum_out=ctxp[:, u:u + 1],
                    )

                ctxn = sm.tile([P, U], F32, tag="ctxn")
                nc.vector.tensor_scalar(
                    out=ctxn, in0=ctxp, scalar1=rs, scalar2=None,
                    op0=mybir.AluOpType.mult,
                )

                # GRU (h0 = 0): gates = ctx_aug.T @ [gk_z|gk_h; gb]
                cT = tps.tile([U, P], F32, tag="tiny_ps")
                nc.tensor.transpose(cT, ctxn, ident_sb)
                caug = sm.tile([KA, P], F32, tag="caug")
                nc.vector.memset(caug, 0.0)
                nc.vector.memset(caug[0:1, :], 1.0)
                nc.vector.tensor_copy(caug[32:48, :], cT)
                gates = tps.tile([P, 2 * U], F32, tag="tiny_ps")
                nc.tensor.matmul(gates, lhsT=caug, rhs=wg_sb,
                                 start=True, stop=True)
                tz = sm.tile([P, U], F32, tag="tz")
                nc.scalar.activation(tz, gates[:, 0:U],
                                     mybir.ActivationFunctionType.Tanh,
                                     scale=0.5)
                th_g = sm.tile([P, U], F32, tag="th_g")
                nc.scalar.activation(th_g, gates[:, U:2 * U],
                                     mybir.ActivationFunctionType.Tanh)
                # hs = (tz - 1) * tanh(xh) = -2 * new_h
                newh = sm.tile([P, U], F32, tag="newh")
                nc.vector.scalar_tensor_tensor(
                    out=newh, in0=tz, scalar=1.0, in1=th_g,
                    op0=mybir.AluOpType.subtract,
                    op1=mybir.AluOpType.mult,
                )

                hT2 = tps.tile([U, P], F32, tag="tiny_ps")
                nc.tensor.transpose(hT2, newh, ident_sb)
                nc.vector.tensor_copy(haug[bt][32:48, :], hT2)
                o_ps = tps.tile([P, O], F32, tag="tiny_ps")
                nc.tensor.matmul(o_ps, lhsT=haug[bt], rhs=wd_sb,
                                 start=True, stop=True)
                nc.vector.tensor_copy(
                    outacc[bt][:, s * O:(s + 1) * O], o_ps)

            # bt-staggered emission: each bt's tail (stream DMA + ctx
            # reduce + GRU) is emitted under the OTHER bt's ACT tanh
            # block, so the ACT FIFO never queues a GRU tanh before a
            # ready tanh block and the enc stream DMA hides fully.
            assert NB == 2
            pend = {}  # bt -> (s, e_sb, rs)
            for s in range(S):
                for bt in range(NB):
                    e_sb, rs = head_phase(s, bt)
                    other = 1 - bt
                    if other in pend:
                        ps, pe, prs = pend.pop(other)
                        tail_phase(ps, other, pe, prs)
                    pend[bt] = (s, e_sb, rs)
            for bt in (0, 1):
                if bt in pend:
                    ps, pe, prs = pend.pop(bt)
                    tail_phase(ps, bt, pe, prs)

            for bt in range(NB):
                nc.sync.dma_start(
                    out=out[bt * P:(bt + 1) * P, :, :].rearrange(
                        "b s o -> b (s o)"),
                    in_=outacc[bt],
                )

    if legalize:
        _legalize_sync_waits(nc)
    return nc


def _pack_weights(w1, w2_k, w2_b, w3_k, gru_k, gru_b, dense_k, dense_b):
    U_ = w1.shape[0]
    w3 = np.asarray(w3_k, np.float32).reshape(U_)
    kron = np.kron(np.eye(P // U_, dtype=np.float16),
                   np.asarray(w1, np.float16))

    # augmented [48, n] weights: row 0 = bias, rows 32:48 = kernel,
    # rows 1:32 = zero. Device h-state is hs = -2*h, so the h-consuming
    # kernels (w2, dense) are scaled by -0.5.
    def aug(kern, bias):
        m = np.zeros((KA, kern.shape[1]), np.float32)
        m[0, :] = bias
        m[32:48, :] = kern
        return m

    wq = aug(np.asarray(w2_k, np.float32) * -0.5, np.asarray(w2_b, np.float32))
    gk = np.asarray(gru_k, np.float32)
    gb = np.asarray(gru_b, np.float32)
    wg = aug(np.hstack([gk[:, 0:U_], gk[:, 2 * U_:3 * U_]]),
             np.hstack([gb[0:U_], gb[2 * U_:3 * U_]]))
    wd = aug(np.asarray(dense_k, np.float32) * -0.5,
             np.asarray(dense_b, np.float32))
    return dict(w3ck=np.broadcast_to(w3.reshape(1, U_),
                                     (P, U_)).astype(np.float32).copy(),
                kronw1=kron, wq=wq, wg=wg, wd=wd,
                ident=np.eye(P, dtype=np.float32))


_PROGRAM_CACHE = {}


def kernel(num_inputs, enc_output, hidden, w1, w2_k, w2_b, w3_k, w3_b,
           gru_k, gru_rk, gru_b, dense_k, dense_b):
    from concourse.bass_utils import run_bass_kernel_spmd

    S = int(num_inputs)
    enc_output = np.asarray(enc_output, np.float32)
    hidden_np = np.asarray(hidden, np.float32)
    B, T, U_ = enc_output.shape
    B_c = B // N_CORES

    key = (B_c, T, S)
    if key not in _PROGRAM_CACHE:
        _PROGRAM_CACHE[key] = build_program(B_c, T, S)
    nc = _PROGRAM_CACHE[key]

    w = _pack_weights(w1, w2_k, w2_b, w3_k, gru_k, gru_b, dense_k, dense_b)

    in_maps = []
    for c in range(N_CORES):
        m = dict(w)
        m["enc"] = enc_output[c * B_c:(c + 1) * B_c]
        # device h-state convention is hs = -2*h
        m["hidden"] = hidden_np[c * B_c:(c + 1) * B_c] * np.float32(-2.0)
        in_maps.append(m)

    res = run_bass_kernel_spmd(nc, in_maps, core_ids=list(range(N_CORES)))
    outs = [res.results[c]["out"].reshape(B_c, S, O) for c in range(N_CORES)]
    return np.concatenate(outs, axis=0).astype(np.float32)


# revision 24
# speedup vs baseline: 1.9489x; 1.1585x over previous
"""Trainium2 Bass kernel for nn_Decoder (Bahdanau-attention decoder scan).

Contract: kernel(**inputs) takes FULL unsharded numpy inputs (keys as in
setup_inputs()) and returns the FULL [B, S, O] float32 output.

Sharding: pure data-parallel over batch B across 8 NeuronCores; weights
replicated; scan carry local per core.

Per-core algorithm (B_c = B/8 = 256, T = 2048, U = 16), v2:
  pre-pass (TileContext #1):
    enc fp32 -> fp16; encw1 = enc @ w1 on PE via a block-diag kron(I8, w1)
    stationary against DMA-xbar-transposed tiles; encw1 lands as two
    resident *u-major* fp16 SBUF tensors [128b, U, T] (contiguous per-u
    planes -> full-rate ACT reads); enc fp16 echo goes to DRAM scratch
    in u-major [b][u][t] layout for per-step streaming (contiguous 4KB
    per-partition DMA reads, packed DVE reads).
  scan (TileContext #2), per step s (bt-staggered emission so the ACT
  FIFO never waits on a GRU tail), per batch tile bt:
    q       = h_aug.T @ [w2_k; w2_b]                    (PE, tiny)
    th_u    = tanh(W[:, u, :] + q[:, u])                (ScalarE, fused
                                                         bias, contiguous)
    score   = sum_u w3[u] * th_u                        (DVE chain of
                                                         scalar_tensor_tensor
                                                         at 4x fp16 rate;
                                                         no PE, no LDWEIGHTS)
    e       = exp(score), sum_e via fused accum_out     (ScalarE; no max-sub
                                                         needed: |score|<~1.3)
    ctx_u   = sum_t e * enc_u / sum_e                   (DVE STT with
                                                         accum_out, per-u
                                                         contiguous planes)
    GRU with h0=0 degenerates:  new_h = 0.5*(1-tanh(xz/2))*tanh(xh)
    (sigmoid avoided => single ACT table set for the whole kernel)
    out_s   = new_h_aug.T @ [dense_k; dense_b]          (PE, tiny)

Two TileContexts: the pre-pass exit barrier resets Tile's vector clocks, so
scan-loop instructions don't inherit one sync-wait per pre-pass DMA HW queue
(walrus allows very few sync waits per datapath instruction).
Resident/state data lives in raw alloc_sbuf_tensor allocations that
survive across the contexts.
"""

import sys

import numpy as np

sys.path.insert(0, "/opt/trn_rl_repo")

import concourse.bass as bass  # noqa: E402
import concourse.tile as tile  # noqa: E402
from concourse import mybir  # noqa: E402

F32 = mybir.dt.float32
F16 = mybir.dt.float16

# Instructions that never carry lowered sync waits / have no events field.
_MULTIWAIT_OK = {"InstUnconditionalBranch", "InstCall",
                 "InstRegisterMove", "InstRegisterAlu"}


def _legalize_sync_waits(nc, max_waits=1):
    """Walrus' codegen allows very few sync-wait commands per datapath
    instruction (matmul LW and TensorScalarPtr fail at 2). Engine queues
    are strict FIFO, so we can splice single-wait NOPs in front of any
    instruction that carries more than `max_waits` waits and leave only
    the last wait on the instruction itself."""
    k = 0
    for func in nc.m.functions:
        for bb in func.blocks:
            il = bb.instructions
            i = 0
            while i < len(il):
                ins = il[i]
                si = ins.sync_info
                if (type(ins).__name__ not in _MULTIWAIT_OK
                        and si is not None and si.on_wait
                        and len(si.on_wait) > max_waits):
                    waits = list(si.on_wait)
                    for w in waits[:-max_waits]:
                        nop = mybir.InstNoOp(name=f"syncsplit-{k}",
                                             ins=[], outs=[])
                        k += 1
                        nop.engine = ins.engine
                        nop.sync_info = mybir.SyncInfo(on_wait=[w],
                                                       on_update=[])
                        il.insert(i, nop)
                        i += 1
                    ins.sync_info = mybir.SyncInfo(
                        on_wait=waits[-max_waits:],
                        on_update=list(si.on_update or []))
                i += 1
    return k


N_CORES = 8
U = 16
O = 8
P = 128  # partitions
KA = 48  # augmented contraction: row 0 = bias, rows 32:48 = data


def build_program(B_c, T, S, legalize=True):
    """Build the single-core bass program (same program runs SPMD per core)."""
    assert B_c % P == 0 and T % P == 0
    NB = B_c // P
    TLO = 8
    NTHI = T // TLO
    CT = min(128, T)  # t-chunk for the fp32 load/convert stage
    AXROWS = 2048
    assert (NTHI * B_c) % AXROWS == 0
    assert (T * U) % AXROWS == 0

    nc = bass.Bass("TRN2", target_bir_lowering=False)

    enc = nc.dram_tensor("enc", [B_c, T, U], F32, kind="ExternalInput").ap()
    hidden = nc.dram_tensor("hidden", [B_c, U], F32, kind="ExternalInput").ap()
    w3ck = nc.dram_tensor("w3ck", [P, U], F32, kind="ExternalInput").ap()
    kronw1 = nc.dram_tensor("kronw1", [P, P], F16, kind="ExternalInput").ap()
    wq = nc.dram_tensor("wq", [KA, U], F32, kind="ExternalInput").ap()
    wg = nc.dram_tensor("wg", [KA, 2 * U], F32, kind="ExternalInput").ap()
    wd = nc.dram_tensor("wd", [KA, O], F32, kind="ExternalInput").ap()
    ident = nc.dram_tensor("ident", [P, P], F32, kind="ExternalInput").ap()
    out = nc.dram_tensor("out", [B_c, S, O], F32, kind="ExternalOutput").ap()

    # DRAM scratch
    tmp = nc.dram_tensor("tmp_bm", [NTHI, B_c, TLO * U], F16).ap()
    w1x = nc.dram_tensor("w1x", [U * T, B_c], F16).ap()  # rows = (u, t)
    encsc = nc.dram_tensor("encsc", [U, B_c, T], F16).ap()  # xbar source
    encT = nc.dram_tensor("encT", [T, U, B_c], F16).ap()  # t-partition form

    # raw SBUF residents (survive across both TileContexts)
    w1res = [nc.alloc_sbuf_tensor(f"w1res{bt}", [P, U, T], F16).ap()
             for bt in range(NB)]
    ident_sb = nc.alloc_sbuf_tensor("ident_r", [P, P], F32).ap()
    idm16 = nc.alloc_sbuf_tensor("idm16_r", [P, P], F16).ap()
    w3_sb = nc.alloc_sbuf_tensor("w3_r", [P, U], F32).ap()
    wq_sb = nc.alloc_sbuf_tensor("wq_r", [KA, U], F32).ap()
    wg_sb = nc.alloc_sbuf_tensor("wg_r", [KA, 2 * U], F32).ap()
    wd_sb = nc.alloc_sbuf_tensor("wd_r", [KA, O], F32).ap()
    haug = [nc.alloc_sbuf_tensor(f"haug{bt}", [KA, P], F32).ap()
            for bt in range(NB)]

    # ================= TileContext 1: pre-pass =================
    with tile.TileContext(nc) as tc:
        with tc.tile_pool(name="pp_psum", bufs=2, space="PSUM") as pp_psum, \
             tc.tile_pool(name="pp_sbuf", bufs=2) as pp:

            # small weights into residents
            nc.sync.dma_start(out=ident_sb, in_=ident)
            nc.vector.tensor_copy(idm16, ident_sb)
            nc.sync.dma_start(out=w3_sb, in_=w3ck)
            nc.sync.dma_start(out=wq_sb, in_=wq)
            nc.sync.dma_start(out=wg_sb, in_=wg)
            nc.sync.dma_start(out=wd_sb, in_=wd)

            kron_sb = pp.tile([P, P], F16, tag="kron")
            nc.sync.dma_start(out=kron_sb, in_=kronw1)

            # h_aug init from `hidden` (host pre-scales hidden by -2)
            for bt in range(NB):
                h0 = pp.tile([P, U], F32, tag="h0")
                nc.sync.dma_start(out=h0, in_=hidden[bt * P:(bt + 1) * P, :])
                hT = pp_psum.tile([U, P], F32, tag="hT")
                nc.tensor.transpose(hT, h0, ident_sb)
                nc.vector.memset(haug[bt], 0.0)
                nc.vector.memset(haug[bt][0:1, :], 1.0)
                nc.vector.tensor_copy(haug[bt][32:48, :], hT)

            # phase 1: fp32 load -> fp16 -> tmp [thi][b][tlo*u]
            # (+ u-major encsc echo via on-chip strided transpose-copy)
            for bt in range(NB):
                for c in range(T // CT):
                    ld = pp.tile([P, CT, U], F32, tag="pp_ld")
                    nc.sync.dma_start(
                        out=ld,
                        in_=enc[bt * P:(bt + 1) * P, c * CT:(c + 1) * CT, :],
                    )
                    cv = pp.tile([P, CT, U], F16, tag="pp_cv")
                    nc.vector.tensor_copy(cv, ld)
                    cvt = pp.tile([P, U, CT], F16, tag="pp_cvt")
                    nc.vector.tensor_copy(
                        cvt, cv.rearrange("b t u -> b u t"))
                    encsc_b = encsc.rearrange("u b t -> b u t")
                    nc.scalar.dma_start(
                        out=encsc_b[bt * P:(bt + 1) * P, :,
                                    c * CT:(c + 1) * CT],
                        in_=cvt,
                    )
                    tv = tmp.rearrange("thi b i -> b thi i")
                    nthi = CT // TLO
                    nc.sync.dma_start(
                        out=tv[bt * P:(bt + 1) * P,
                               c * nthi:(c + 1) * nthi, :],
                        in_=cv.rearrange("b (thi tlo) u -> b thi (tlo u)",
                                         tlo=TLO),
                    )

            # phase 2: A-xbar transpose + blockdiag matmul -> w1x [(u t), b]
            # (u-major row order so phase 3 lands contiguous u-planes).
            # evbig partitions are (tlo, v) but w1x rows are (v, thi, tlo),
            # which is not AP-expressible in one go -- write one DMA per
            # tlo slice (16 v-partitions each), alternating trigger queues.
            tmp_rows = tmp.rearrange("thi b i -> (thi b) i")
            w1_v = w1x.rearrange("(v thi tlo) b -> tlo v thi b",
                                 v=U, tlo=TLO)
            nax = (NTHI * B_c) // AXROWS
            thi_per_ax = AXROWS // B_c
            for r in range(nax):
                ax = pp.tile([P, AXROWS], F16, tag="pp_ax")
                nc.sync.dma_start(
                    out=ax,
                    in_=tmp_rows[r * AXROWS:(r + 1) * AXROWS, :],
                    transpose=True,
                )
                evbig = pp.tile([P, AXROWS], F16, tag="pp_ev")
                nmm = AXROWS // 512
                for n in range(nmm):
                    ps = pp_psum.tile([P, 512], F32, tag="pp_bd")
                    nc.tensor.matmul(ps, lhsT=kron_sb,
                                     rhs=ax[:, n * 512:(n + 1) * 512],
                                     start=True, stop=True)
                    if n % 2 == 0:
                        nc.vector.tensor_copy(
                            evbig[:, n * 512:(n + 1) * 512], ps)
                    else:
                        nc.scalar.copy(evbig[:, n * 512:(n + 1) * 512], ps)
                ts = slice(r * thi_per_ax, (r + 1) * thi_per_ax)
                ev3 = evbig.rearrange("p (thi b) -> p thi b",
                                      thi=thi_per_ax)
                for c in range(TLO):
                    eng = nc.sync if c % 2 == 0 else nc.scalar
                    eng.dma_start(
                        out=w1_v[c, :, ts, :],
                        in_=ev3[c * U:(c + 1) * U, :, :],
                    )

            # phase 1.5: second xbar pass encsc [(u b), t] -> encT [t, (u b)]
            # (gives the scan a t-partition form of enc so the ctx reduce
            # can run on PE as a transposed-e matmul). On the ACT trigger
            # queue to keep it off the SP xbar queue's critical path.
            encsc_rows = encsc.rearrange("u b t -> (u b) t")
            encT_rows = encT.rearrange("t u b -> t (u b)")
            for j in range(T // P):
                tt = pp.tile([P, U * B_c], F16, tag="pp_tt")
                nc.scalar.dma_start(
                    out=tt,
                    in_=encsc_rows[:, j * P:(j + 1) * P],
                    transpose=True,
                )
                nc.scalar.dma_start(
                    out=encT_rows[j * P:(j + 1) * P, :],
                    in_=tt,
                )

            # phase 3: B-xbar -> resident u-major encw1 tensors
            for bt in range(NB):
                plv = w1res[bt].rearrange("b u t -> b (u t)")
                for r in range((T * U) // AXROWS):
                    nc.sync.dma_start(
                        out=plv[:, r * AXROWS:(r + 1) * AXROWS],
                        in_=w1x[r * AXROWS:(r + 1) * AXROWS,
                                bt * P:(bt + 1) * P],
                        transpose=True,
                    )

    TH = T // 2   # score-chain half length (DVE half / GpSimd half)
    NC_ = T // P  # t-chunks for the ctx matmul

    # ================= TileContext 2: the decoder scan =================
    with tile.TileContext(nc) as tc2:
        with tc2.tile_pool(name="ctx_psum", bufs=1, space="PSUM") as cps, \
             tc2.tile_pool(name="tr_psum", bufs=2, space="PSUM") as trps, \
             tc2.tile_pool(name="tiny_psum", bufs=2, space="PSUM") as tps, \
             tc2.tile_pool(name="planes", bufs=3) as planes, \
             tc2.tile_pool(name="scoreb", bufs=2) as scoreb, \
             tc2.tile_pool(name="stream", bufs=4) as stream, \
             tc2.tile_pool(name="etp", bufs=4) as etp, \
             tc2.tile_pool(name="sm", bufs=2) as sm, \
             tc2.tile_pool(name="junk", bufs=2) as junkp, \
             tc2.tile_pool(name="outp", bufs=1) as outp:

            outacc = []
            for bt in range(NB):
                oa = outp.tile([P, S * O], F32, tag=f"outacc{bt}",
                               name=f"outacc{bt}")
                outacc.append(oa)

            def head_phase(s, bt):
                """q -> 16x tanh -> w3-weighted score chains (DVE on the
                low t-half, GpSimd on the high t-half, both chasing ACT)
                -> exp per half. Returns (e0, e1, rs)."""
                q_ps = tps.tile([P, U], F32, tag="tiny_ps")
                nc.tensor.matmul(q_ps, lhsT=haug[bt], rhs=wq_sb,
                                 start=True, stop=True)
                q_sb = sm.tile([P, U], F32, tag="q_sb")
                nc.vector.tensor_copy(q_sb, q_ps)

                sc_prev = None
                for u in range(U):
                    th = planes.tile([P, T], F16, tag="tanh_plane")
                    nc.scalar.activation(
                        th, w1res[bt][:, u, :],
                        mybir.ActivationFunctionType.Tanh,
                        bias=q_sb[:, u:u + 1], scale=1.0,
                    )
                    sc = scoreb.tile([P, T], F16, tag="score")
                    if u == 0:
                        nc.vector.tensor_scalar(
                            out=sc, in0=th, scalar1=w3_sb[:, 0:1],
                            scalar2=None, op0=mybir.AluOpType.mult,
                        )
                    else:
                        # TS (4x) + plain TT add (2x) beats the fused STT
                        # form, which measures 1x on this hardware
                        tsx = planes.tile([P, T], F16, tag="ts_probe")
                        nc.vector.tensor_scalar(
                            out=tsx, in0=th, scalar1=w3_sb[:, u:u + 1],
                            scalar2=None, op0=mybir.AluOpType.mult,
                        )
                        nc.vector.tensor_add(sc, tsx, sc_prev)
                    sc_prev = sc

                e_sb = sm.tile([P, T], F16, tag="e_sb")
                sum_e = sm.tile([P, 1], F32, tag="sum_e")
                nc.scalar.activation(
                    e_sb, sc_prev, mybir.ActivationFunctionType.Exp,
                    accum_out=sum_e,
                )
                rs = sm.tile([P, 1], F32, tag="rs")
                nc.vector.reciprocal(rs, sum_e)
                return e_sb, rs

            def tail_phase(s, bt, e_sb, rs):
                """ctx reduce on PE: stream encT t-chunks, transpose e
                per chunk, accumulate eT.T @ encT into PSUM [b, (u, b')],
                extract the b'=b diagonal per u via tiny masked STTs."""
                bsl = slice(bt * P, (bt + 1) * P)
                ctx_ps = cps.tile([P, U * P], F32, tag="ctx_ps")
                for c in range(NC_):
                    ec = stream.tile([P, U, P], F16, tag="ec")
                    nc.sync.dma_start(
                        out=ec, in_=encT[c * P:(c + 1) * P, :, bsl],
                    )
                    psT = trps.tile([P, P], F16, tag="psT")
                    nc.tensor.transpose(
                        psT, e_sb[:, c * P:(c + 1) * P], idm16)
                    eTt = etp.tile([P, P], F16, tag="eTt")
                    nc.vector.tensor_copy(eTt, psT)
                    rhs = ec.rearrange("t u b -> t (u b)")
                    for q in range(U * P // 512):
                        nc.tensor.matmul(
                            ctx_ps[:, q * 512:(q + 1) * 512],
                            lhsT=eTt, rhs=rhs[:, q * 512:(q + 1) * 512],
                            start=(c == 0), stop=(c == NC_ - 1),
                        )

                ctxp = sm.tile([P, U], F32, tag="ctxp")
                for u in range(U):
                    junk = junkp.tile([P, P], F16, tag="junk")
                    nc.vector.scalar_tensor_tensor(
                        out=junk, in0=ctx_ps[:, u * P:(u + 1) * P],
                        scalar=1.0, in1=idm16,
                        op0=mybir.AluOpType.mult,
                        op1=mybir.AluOpType.mult,
                        accum_out=ctxp[:, u:u + 1],
                    )

                ctxn = sm.tile([P, U], F32, tag="ctxn")
                nc.vector.tensor_scalar(
                    out=ctxn, in0=ctxp, scalar1=rs, scalar2=None,
                    op0=mybir.AluOpType.mult,
                )

                # GRU (h0 = 0): gates = ctx_aug.T @ [gk_z|gk_h; gb]
                cT = tps.tile([U, P], F32, tag="tiny_ps")
                nc.tensor.transpose(cT, ctxn, ident_sb)
                caug = sm.tile([KA, P], F32, tag="caug")
                nc.vector.memset(caug, 0.0)
                nc.vector.memset(caug[0:1, :], 1.0)
                nc.vector.tensor_copy(caug[32:48, :], cT)
                gates = tps.tile([P, 2 * U], F32, tag="tiny_ps")
                nc.tensor.matmul(gates, lhsT=caug, rhs=wg_sb,
                                 start=True, stop=True)
                tz = sm.tile([P, U], F32, tag="tz")
                nc.scalar.activation(tz, gates[:, 0:U],
                                     mybir.ActivationFunctionType.Tanh,
                                     scale=0.5)
                th_g = sm.tile([P, U], F32, tag="th_g")
                nc.scalar.activation(th_g, gates[:, U:2 * U],
                                     mybir.ActivationFunctionType.Tanh)
                # hs = (tz - 1) * tanh(xh) = -2 * new_h
                newh = sm.tile([P, U], F32, tag="newh")
                nc.vector.scalar_tensor_tensor(
                    out=newh, in0=tz, scalar=1.0, in1=th_g,
                    op0=mybir.AluOpType.subtract,
                    op1=mybir.AluOpType.mult,
                )

                hT2 = tps.tile([U, P], F32, tag="tiny_ps")
                nc.tensor.transpose(hT2, newh, ident_sb)
                nc.vector.tensor_copy(haug[bt][32:48, :], hT2)
                o_ps = tps.tile([P, O], F32, tag="tiny_ps")
                nc.tensor.matmul(o_ps, lhsT=haug[bt], rhs=wd_sb,
                                 start=True, stop=True)
                nc.vector.tensor_copy(
                    outacc[bt][:, s * O:(s + 1) * O], o_ps)

            # bt-staggered emission: each bt's tail (stream DMA + ctx
            # reduce + GRU) is emitted under the OTHER bt's ACT tanh
            # block, so the ACT FIFO never queues a GRU tanh before a
            # ready tanh block and the enc stream DMA hides fully.
            assert NB == 2
            pend = {}  # bt -> (s, e_sb, rs)
            for s in range(S):
                for bt in range(NB):
                    e_sb, rs = head_phase(s, bt)
                    other = 1 - bt
                    if other in pend:
                        ps, pe, prs = pend.pop(other)
                        tail_phase(ps, other, pe, prs)
                    pend[bt] = (s, e_sb, rs)
            for bt in (0, 1):
                if bt in pend:
                    ps, pe, prs = pend.pop(bt)
                    tail_phase(ps, bt, pe, prs)

            for bt in range(NB):
                nc.sync.dma_start(
                    out=out[bt * P:(bt + 1) * P, :, :].rearrange(
                        "b s o -> b (s o)"),
                    in_=outacc[bt],
                )

    if legalize:
        _legalize_sync_waits(nc)
    return nc


def _pack_weights(w1, w2_k, w2_b, w3_k, gru_k, gru_b, dense_k, dense_b):
    U_ = w1.shape[0]
    w3 = np.asarray(w3_k, np.float32).reshape(U_)
    kron = np.kron(np.eye(P // U_, dtype=np.float16),
                   np.asarray(w1, np.float16))

    # augmented [48, n] weights: row 0 = bias, rows 32:48 = kernel,
    # rows 1:32 = zero. Device h-state is hs = -2*h, so the h-consuming
    # kernels (w2, dense) are scaled by -0.5.
    def aug(kern, bias):
        m = np.zeros((KA, kern.shape[1]), np.float32)
        m[0, :] = bias
        m[32:48, :] = kern
        return m

    wq = aug(np.asarray(w2_k, np.float32) * -0.5, np.asarray(w2_b, np.float32))
    gk = np.asarray(gru_k, np.float32)
    gb = np.asarray(gru_b, np.float32)
    wg = aug(np.hstack([gk[:, 0:U_], gk[:, 2 * U_:3 * U_]]),
             np.hstack([gb[0:U_], gb[2 * U_:3 * U_]]))
    wd = aug(np.asarray(dense_k, np.float32) * -0.5,
             np.asarray(dense_b, np.float32))
    return dict(w3ck=np.broadcast_to(w3.reshape(1, U_),
                                     (P, U_)).astype(np.float32).copy(),
                kronw1=kron, wq=wq, wg=wg, wd=wd,
                ident=np.eye(P, dtype=np.float32))


_PROGRAM_CACHE = {}


def kernel(num_inputs, enc_output, hidden, w1, w2_k, w2_b, w3_k, w3_b,
           gru_k, gru_rk, gru_b, dense_k, dense_b):
    from concourse.bass_utils import run_bass_kernel_spmd

    S = int(num_inputs)
    enc_output = np.asarray(enc_output, np.float32)
    hidden_np = np.asarray(hidden, np.float32)
    B, T, U_ = enc_output.shape
    B_c = B // N_CORES

    key = (B_c, T, S)
    if key not in _PROGRAM_CACHE:
        _PROGRAM_CACHE[key] = build_program(B_c, T, S)
    nc = _PROGRAM_CACHE[key]

    w = _pack_weights(w1, w2_k, w2_b, w3_k, gru_k, gru_b, dense_k, dense_b)

    in_maps = []
    for c in range(N_CORES):
        m = dict(w)
        m["enc"] = enc_output[c * B_c:(c + 1) * B_c]
        # device h-state convention is hs = -2*h
        m["hidden"] = hidden_np[c * B_c:(c + 1) * B_c] * np.float32(-2.0)
        in_maps.append(m)

    res = run_bass_kernel_spmd(nc, in_maps, core_ids=list(range(N_CORES)))
    outs = [res.results[c]["out"].reshape(B_c, S, O) for c in range(N_CORES)]
    return np.concatenate(outs, axis=0).astype(np.float32)
